# revision 1
# baseline (speedup 1.0000x reference)
"""Trainium2 Bass kernel: data-dependent radix-2 FFT butterfly network.

out = FFT-like transform of x (4096x4096 f32 -> complex64); stage twiddles
are exp(-2j*pi*k/N * weights[k, :]) (learned, per-feature), N = 4096,
12 radix-2 stages, initial row permutation j ^ N/2.

Sharding: feature dim split across 8 NeuronCores (512 each) - the whole
network is elementwise along features, so no cross-core communication.

Per-core: features on partitions (4 groups of 128), FFT rows along the
free dim, x stored as packed (re, im) fp16 pairs. Each generic stage is
3 Vector-engine ops: a hand-written packed-complex-multiply custom DVE
op (one complex/cycle in 2X_1PORT mode; uop program written directly at
the UopConfig level with a hand-rolled 2x table slot) plus two packed
fp16 add/subs in 2x mode. Stages 1-2 (real inputs, trivial twiddles) are
fused into three custom quad ops. Twiddles are generated on-device by
the Scalar engine's Sin LUT from host-range-reduced phases; the Scalar
engine also assembles packed layouts and the final fp32 output, fully
overlapped with Vector work. The host pre-transposes inputs so all DMAs
are contiguous and un-transposes the [feat, (row, re/im)] output.

Measured: ~365 us HW exec for the full 8-core transform, rel err ~1e-3.
"""

import math
import sys

import numpy as np

if "/opt/trn_rl_repo" not in sys.path:
    sys.path.insert(0, "/opt/trn_rl_repo")

import concourse.bacc as bacc
import concourse.bass as bass
import concourse.mybir as mybir
from concourse.bass_utils import run_bass_kernel_spmd
from concourse.tile import TileContext

F32 = mybir.dt.float32
F16 = mybir.dt.float16
AF = mybir.ActivationFunctionType
ALU = mybir.AluOpType

N = 4096
LOGN = 12
NCORES = 8
DSH = N // NCORES
NGROUPS = DSH // 128
PI = math.pi
TWO_PI = 2.0 * math.pi


# ===================== custom DVE ops =====================

import numpy as np

import concourse.dve_ops as dve_ops
from concourse.dve_spec import Spec, Src0, Src1
from concourse.dve_uop import (
    AluInp,
    AluOp,
    DelayInp,
    DveOpSpec,
    InpSel,
    OutPath,
    OutSel,
    Trigger,
    UopConfig,
)

D = [
    AluInp.PREV_DELAY_0,
    AluInp.PREV_DELAY_1,
    AluInp.PREV_DELAY_2,
    AluInp.PREV_DELAY_3,
    AluInp.PREV_DELAY_4,
    AluInp.PREV_DELAY_5,
]


def _uop(inputs, req0, req1, trigger, next_uop, repeat=0):
    u = UopConfig()
    for lane, sel in enumerate(inputs, start=1):
        u.enable_input(sel, lane)
    u.require_inp0 = req0
    u.require_inp1 = req1
    u.trigger = trigger
    u.next_uop = next_uop
    u.repeat_count = repeat
    return u


# ---------------- CMUL (1-state, proven) ----------------


def _cmul_uop():
    u = _uop(
        [InpSel.SRC_0, InpSel.SRC_1, InpSel.SRC_0_HI, InpSel.SRC_1_HI],
        1,
        1,
        (Trigger.SRC_TENSOR_DONE, Trigger.NONE, Trigger.NONE),
        (0, 0, 0),
    )
    dp = u.datapath_config
    dp[0].enable_alu(AluOp.MULTIPLY, D[0], D[1])
    dp[0].pass_through_delay(0, 1, 2, 3)
    dp[1].enable_alu(AluOp.MULTIPLY, D[2], D[3])
    dp[1].pass_through_delay(0, 1, 2, 3)
    dp[1].enable_delay_from_src(DelayInp.PREV_ALU_OUT, 4)
    dp[2].enable_alu(AluOp.SUBTRACT, D[4], AluInp.PREV_ALU_OUT)
    dp[2].pass_through_delay(0, 1, 2, 3)
    dp[3].enable_alu(AluOp.MULTIPLY, D[0], D[3])
    dp[3].pass_through_delay(1, 2)
    dp[3].enable_delay_from_src(DelayInp.PREV_ALU_OUT, 4)
    dp[4].enable_alu(AluOp.MULTIPLY, D[2], D[1])
    dp[4].pass_through_delay(4)
    dp[4].enable_delay_from_src(DelayInp.PREV_ALU_OUT, 0)
    dp[5].enable_alu(AluOp.ADD, D[0], AluInp.PREV_ALU_OUT)
    dp[5].pass_through_delay(4)
    dp[6].pass_through_alu()
    dp[6].pass_through_delay(4)
    dp[7].pass_through_alu()
    dp[7].pass_through_delay(4)
    u.enable_output(OutSel.DELAY_4, OutPath.WR0_LO)
    u.enable_output(OutSel.ALU_OUT, OutPath.WR0_HI)
    return u


def _cmul_reference(in0, in1, c0, c1, c2):
    a = in0.astype(np.float32)
    b = np.broadcast_to(in1, in0.shape).astype(np.float32)
    out = np.empty_like(a)
    ar, ai = a[..., 0::2], a[..., 1::2]
    br, bi = b[..., 0::2], b[..., 1::2]
    out[..., 0::2] = ar * br - ai * bi
    out[..., 1::2] = ar * bi + ai * br
    return out


# ---------------- stage-1+2 fused single-state ops ----------------
# src0 = even fp16 pairs (a,b) of each 4-row quad, src1 = odd pairs (c,d),
# CONST_0 = tw component. Let t0=a+b, t2=a-b, t1=c+d, u=c-d, m=C0*u.
#   S2A  -> (y0, y1) = (t0+t1, t2+m)   [even pair slots of the out plane]
#   S2B  -> (y2, y3) = (t0-t1, t2-m)   [odd pair slots]
#   S2IM -> (m, -m)                    [compact; ACT scatters to im slots]

_S2_IN = [InpSel.SRC_0, InpSel.SRC_0_HI, InpSel.SRC_1, InpSel.SRC_1_HI, InpSel.CONST_0]


def _s2_uop(sub: bool):
    u = _uop(
        _S2_IN,
        1,
        1,
        (Trigger.SRC_TENSOR_DONE, Trigger.NONE, Trigger.NONE),
        (0, 0, 0),
    )
    dp = u.datapath_config
    dp[0].enable_alu(AluOp.ADD, D[0], D[1])  # t0 = a+b
    dp[0].pass_through_delay(0, 1, 2, 3, 4)
    dp[1].enable_alu(AluOp.ADD, D[2], D[3])  # t1 = c+d
    dp[1].enable_delay_from_src(DelayInp.PREV_ALU_OUT, 5)  # t0
    dp[1].pass_through_delay(0, 1, 2, 3, 4)
    dp[2].enable_alu(AluOp.SUBTRACT, D[0], D[1])  # t2 = a-b
    dp[2].enable_delay_from_src(DelayInp.PREV_ALU_OUT, 0)  # t1
    dp[2].pass_through_delay(2, 3, 4, 5)
    dp[3].enable_alu(AluOp.SUBTRACT, D[2], D[3])  # u = c-d
    dp[3].enable_delay_from_src(DelayInp.PREV_ALU_OUT, 1)  # t2
    dp[3].pass_through_delay(0, 4, 5)
    dp[4].enable_alu(AluOp.MULTIPLY, AluInp.PREV_ALU_OUT, D[4])  # m = u*C0
    dp[4].pass_through_delay(0, 1, 5)
    op = AluOp.SUBTRACT if sub else AluOp.ADD
    dp[5].enable_alu(op, D[5], D[0])  # y0/y2 = t0 (+/-) t1
    dp[5].enable_delay_from_src(DelayInp.PREV_ALU_OUT, 2)  # m
    dp[5].pass_through_delay(1)
    dp[6].enable_alu(op, D[1], D[2])  # y1/y3 = t2 (+/-) m
    dp[6].enable_delay_from_src(DelayInp.PREV_ALU_OUT, 3)  # y0/y2
    dp[7].pass_through_alu()  # y1/y3 rides ALU
    dp[7].pass_through_delay(3)
    u.enable_output(OutSel.DELAY_3, OutPath.WR0_LO)
    u.enable_output(OutSel.ALU_OUT, OutPath.WR0_HI)
    return u


def _s2a_reference(in0, in1, c0, c1, c2):
    a = in0.astype(np.float32)
    b = np.asarray(in1).astype(np.float32)
    c = np.asarray(c0, np.float32).reshape(-1, *([1] * (a.ndim - 1)))
    t0 = a[..., 0::2] + a[..., 1::2]
    t2 = a[..., 0::2] - a[..., 1::2]
    t1 = b[..., 0::2] + b[..., 1::2]
    m = c * (b[..., 0::2] - b[..., 1::2])
    out = np.empty_like(a)
    out[..., 0::2] = t0 + t1
    out[..., 1::2] = t2 + m
    return out


def _s2b_reference(in0, in1, c0, c1, c2):
    a = in0.astype(np.float32)
    b = np.asarray(in1).astype(np.float32)
    c = np.asarray(c0, np.float32).reshape(-1, *([1] * (a.ndim - 1)))
    t0 = a[..., 0::2] + a[..., 1::2]
    t2 = a[..., 0::2] - a[..., 1::2]
    t1 = b[..., 0::2] + b[..., 1::2]
    m = c * (b[..., 0::2] - b[..., 1::2])
    out = np.empty_like(a)
    out[..., 0::2] = t0 - t1
    out[..., 1::2] = t2 - m
    return out


_S2IM_IN = [
    InpSel.SRC_0,
    InpSel.SRC_0_HI,
    InpSel.SRC_1,
    InpSel.SRC_1_HI,
    InpSel.CONST_0,
    InpSel.ZERO,
]


def _s2im_uop():
    u = _uop(
        _S2IM_IN,
        1,
        1,
        (Trigger.SRC_TENSOR_DONE, Trigger.NONE, Trigger.NONE),
        (0, 0, 0),
    )
    dp = u.datapath_config
    dp[0].enable_alu(AluOp.SUBTRACT, D[2], D[3])  # u = c-d
    dp[0].pass_through_delay(4, 5)
    dp[1].enable_alu(AluOp.MULTIPLY, AluInp.PREV_ALU_OUT, D[4])  # m
    dp[1].pass_through_delay(5)
    dp[2].enable_alu(AluOp.SUBTRACT, D[5], AluInp.PREV_ALU_OUT)  # -m
    dp[2].enable_delay_from_src(DelayInp.PREV_ALU_OUT, 0)  # m
    for k in (3, 4, 5, 6, 7):
        dp[k].pass_through_alu()
        dp[k].pass_through_delay(0)
    u.enable_output(OutSel.DELAY_0, OutPath.WR0_LO)  # m
    u.enable_output(OutSel.ALU_OUT, OutPath.WR0_HI)  # -m
    return u


def _s2im_reference(in0, in1, c0, c1, c2):
    b = np.asarray(in1).astype(np.float32)
    s = np.asarray(c0, np.float32).reshape(-1, *([1] * (b.ndim - 1)))
    m = s * (b[..., 0::2] - b[..., 1::2])
    out = np.empty_like(b)
    out[..., 0::2] = m
    out[..., 1::2] = -m
    return out


# ---------------- registry ----------------


class RawDveOp:
    def __init__(self, name, mk_uops, rd1_en, perf_max, reference):
        self.name = name
        self.subdim = False
        self.spec = Spec(body=Src0 * Src1 if rd1_en else Src0, reference=reference)
        self.rd1_en = rd1_en
        self.perf_max = perf_max
        self._mk = mk_uops
        self._cache = {}

    def compile(self, ver):
        if ver in self._cache:
            return self._cache[ver]
        uops = self._mk()
        spec = DveOpSpec(
            name=self.name,
            uops=[u for u in uops],
            opcode=dve_ops.get_dve_sub_opcode(self.name),
            uops_2x=self._mk(),
            perf_max=self.perf_max,
            rd1_en=self.rd1_en,
        )
        spec.validate(ver)
        self._cache[ver] = spec
        return spec


RAW_OPS = {}


def register_raw_ops():
    if RAW_OPS:
        return RAW_OPS
    defs = [
        RawDveOp("CMUL_PACKED_ANT", lambda: [_cmul_uop()], True, 1, _cmul_reference),
        RawDveOp("S2A_ANT", lambda: [_s2_uop(False)], True, 1, _s2a_reference),
        RawDveOp("S2B_ANT", lambda: [_s2_uop(True)], True, 1, _s2b_reference),
        RawDveOp("S2IM_ANT", lambda: [_s2im_uop()], True, 1, _s2im_reference),
    ]
    for op in defs:
        if op.name not in dve_ops._SUB_OPCODE_FOR_NAME:
            dve_ops.OPS.append(op)
            row = dve_ops._CUSTOM_DVE_ROW_BASE + len(dve_ops.OPS) - 1
            assert row < 0x20
            dve_ops._SUB_OPCODE_FOR_NAME[op.name] = row
            dve_ops.CUSTOM_DVE_SPECS[op.name] = op.spec
        RAW_OPS[op.name] = op
    return RAW_OPS


def emit_raw(nc, name, out, in0, in1=None, s0=None):
    import concourse.bass_isa as bass_isa
    import concourse.mybir as mybir

    ops = register_raw_ops()
    op = ops[name]
    v = nc.vector
    if op.name not in nc.m.ant_custom_dve_ops:
        nc.m.ant_custom_dve_ops = sorted({*nc.m.ant_custom_dve_ops, op.name})
    shape = (
        bass_isa.CustomDveShape.STT
        if in1 is not None
        else bass_isa.CustomDveShape.TTSS
    )
    isa_opcode = nc.isa.Opcode[
        f"NEURON_ISA_TPB_OPCODE_CUSTOM_DVE_ANT_{shape.slot()}"
    ].value
    imm = mybir.ImmediateValue(dtype=mybir.dt.float32, value=0.0)
    s0a = v.lower_ap(s0, for_isa=True) if s0 is not None else imm
    ins = [v.lower_ap(in0, for_isa=True)]
    if in1 is not None:
        ins.append(v.lower_ap(in1, for_isa=True))
    ins += [s0a, imm]
    return v.add_instruction(
        bass_isa.InstCustomDveAnt(
            name=nc.get_next_instruction_name(),
            op_name=op.name,
            rd1_en=op.rd1_en,
            subdim=0,
            imm2=0.0,
            shape=shape,
            row=dve_ops.get_dve_sub_opcode(op.name),
            isa_opcode=isa_opcode,
            ins=ins,
            outs=[v.lower_ap(out, for_isa=True)],
        )
    )


def patch_perf_bits(nc):
    ops = register_raw_ops()
    n = 0
    for fn in nc.m.functions:
        for blk in fn.blocks:
            for inst in blk.instructions:
                nm = getattr(inst, "op_name", None)
                if nm in ops:
                    bb = bytearray(bytes(inst.instr))
                    bb[36] |= ops[nm].perf_max << 6
                    inst.instr = bytes(bb)
                    n += 1
    return n


# ===================== kernel builder =====================

def build_fft_nc():
    register_raw_ops()
    nc = bacc.Bacc()

    xT = nc.dram_tensor("xT", [DSH, N], F32, kind="ExternalInput")
    wT = nc.dram_tensor("wT", [DSH, N // 2], F32, kind="ExternalInput")
    outT = nc.dram_tensor("outT", [DSH, 2 * N], F32, kind="ExternalOutput")

    # const AP: pi/2 bias for the cos path
    HPI = float(np.float32(PI / 2))
    chp = nc.alloc_sbuf_tensor("const-f32-hpi", [128, 1], F32)
    nc.gpsimd.memset(chp.ap(), HPI)
    nc.const_aps.aps[(F32, HPI)] = chp.ap()
    nc.all_engine_barrier()

    with TileContext(nc) as tc:
        with (
            tc.tile_pool(name="const", bufs=1) as cpool,
            tc.tile_pool(name="wld", bufs=2) as wpool,
            tc.tile_pool(name="tw", bufs=1) as twpool,
            tc.tile_pool(name="xbuf", bufs=2) as xpool,
            tc.tile_pool(name="tmp", bufs=1) as tpool,
            tc.tile_pool(name="ph", bufs=1) as ppool,
            tc.tile_pool(name="obuf", bufs=2) as opool,
        ):
            for g in range(NGROUPS):
                rows = slice(g * 128, (g + 1) * 128)

                # ---- load x first: contiguous fp16 re-plane (half-swap) ----
                xr = tpool.tile([128, N], F16, tag="xplane")
                nc.gpsimd.dma_start(xr[:, 0 : N // 2], xT[rows, N // 2 : N])
                nc.gpsimd.dma_start(xr[:, N // 2 : N], xT[rows, 0 : N // 2])

                # ---- phases arrive host-reduced: wT[p,k] = r_red in
                # [-0.5, 0.5] with sin(2pi*r_red) = sin(phi).
                # cos(phi) = cos(2pi*|r_red|) = sin(pi/2 - 2pi*|r_red|) ----
                r = ppool.tile([128, N // 2], F32, tag="r")
                nc.sync.dma_start(r[:], wT[rows, :])
                absr = ppool.tile([128, N // 2], F32, tag="absr")
                nc.scalar.activation(absr[:], r[:], AF.Abs)

                # ---- interleaved twiddle packs: stage s at [2*half, 4*half) ----
                pack = twpool.tile([128, 2 * N], F16, tag="pack")
                for s in range(3, LOGN + 1):
                    half = 1 << (s - 1)
                    stride = N >> s
                    src_im = r[:, 0 : N // 2 : stride]
                    src_re = absr[:, 0 : N // 2 : stride]
                    nc.scalar.activation(
                        pack[:, 2 * half : 4 * half : 2],
                        src_re,
                        AF.Sin,
                        scale=-TWO_PI,
                        bias=HPI,
                    )
                    nc.scalar.activation(
                        pack[:, 2 * half + 1 : 4 * half : 2],
                        src_im,
                        AF.Sin,
                        scale=TWO_PI,
                    )
                # stage-2 twiddle columns (c, s) for k=1024, fp32 [P,1]
                c2 = ppool.tile([128, 2], F32, tag="cols")
                nc.scalar.activation(
                    c2[:, 0:1], absr[:, 1024:1025], AF.Sin, scale=-TWO_PI, bias=HPI
                )
                nc.scalar.activation(
                    c2[:, 1:2], r[:, 1024:1025], AF.Sin, scale=TWO_PI
                )

                x = xpool.tile([128, 2 * N], F16, tag="x")  # packed (re, im)
                t1 = tpool.tile([128, N], F16, tag="t1")  # packed cmul temp

                # ---- stages 1+2 fused: three single-state custom ops over
                # raw 4-row quads (a,b,c,d) of the re-plane ----
                nc.gpsimd.memset(x[:], 0.0)
                xr4 = xr[:].rearrange("p (b f) -> p b f", f=4)
                src0 = xr4[:, :, 0:2]
                src1 = xr4[:, :, 2:4]
                pre = tpool.tile([128, N], F16, tag="pre")
                pre4 = pre[:].rearrange("p (b f) -> p b f", f=4)
                imc = tpool.tile([128, N // 2], F16, tag="imc")
                imc2 = imc[:].rearrange("p (b f) -> p b f", f=2)
                emit_raw(nc, "S2A_ANT", pre4[:, :, 0:2], src0, src1, s0=c2[:, 0:1])
                emit_raw(nc, "S2B_ANT", pre4[:, :, 2:4], src0, src1, s0=c2[:, 0:1])
                emit_raw(nc, "S2IM_ANT", imc2, src0, src1, s0=c2[:, 1:2])
                # scatter to packed complex x (ACT): re at even slots, the
                # (m,-m) pairs at im slots of rows 4b+1 / 4b+3
                nc.scalar.activation(x[:, 0 : 2 * N : 2], pre[:], AF.Copy)
                x8 = x[:].rearrange("p (b f) -> p b f", f=8)
                nc.scalar.activation(x8[:, :, 3:8:4], imc2, AF.Copy)

                # ---- stages 3..11: packed generic ----
                for s in range(3, LOGN):
                    step = 1 << s
                    half = step // 2
                    nb = N // step

                    xv = x[:].rearrange("p (b stc) -> p b stc", stc=2 * step)
                    top = xv[:, :, 0 : 2 * half]
                    bot = xv[:, :, 2 * half : 2 * step]
                    tw = (
                        pack[:, 2 * half : 4 * half]
                        .unsqueeze(1)
                        .broadcast_to([128, nb, 2 * half])
                    )
                    tv = t1[:, 0 : nb * 2 * half]
                    if nb > 1:
                        tv = tv.rearrange("p (b h) -> p b h", h=2 * half)
                    emit_raw(nc, "CMUL_PACKED_ANT", tv, bot, tw)
                    nc.vector.tensor_sub(bot, top, tv)
                    nc.vector.tensor_add(top, top, tv)

                # ---- stage 12 in two column chunks; each chunk's outputs
                # (one bottom + one top quarter) drain to HBM immediately ----
                Q = N // 2
                H12 = N // 2  # half = 2048 rows; packed column width N
                for c in range(2):
                    csl = slice(c * Q, c * Q + Q)
                    top_c = x[:, c * Q : c * Q + Q]
                    bot_c = x[:, N + c * Q : N + c * Q + Q]
                    tw_c = pack[:, N + c * Q : N + c * Q + Q]
                    tv = t1[:, 0:Q]
                    emit_raw(nc, "CMUL_PACKED_ANT", tv, bot_c, tw_c)
                    nc.vector.tensor_sub(bot_c, top_c, tv)
                    hq = 2 + c
                    o_b = opool.tile([128, Q], F32, tag="out")
                    nc.scalar.activation(o_b[:], x[:, hq * Q : (hq + 1) * Q], AF.Copy)
                    nc.sync.dma_start(outT[rows, hq * Q : (hq + 1) * Q], o_b[:])
                    nc.vector.tensor_add(top_c, top_c, tv)
                    o_t = opool.tile([128, Q], F32, tag="out")
                    nc.scalar.activation(o_t[:], x[:, c * Q : (c + 1) * Q], AF.Copy)
                    nc.sync.dma_start(outT[rows, c * Q : (c + 1) * Q], o_t[:])

    nc.compile()
    patch_perf_bits(nc)
    return nc


def make_core_inputs(x: np.ndarray, weights: np.ndarray, core: int):
    sl = slice(core * DSH, (core + 1) * DSH)
    xT = np.ascontiguousarray(x[:, sl].T).astype(np.float32, copy=False)
    w = weights[: N // 2, sl].astype(np.float64)
    k = -(1.0 / N) * np.arange(N // 2, dtype=np.float64)
    rr = w * k[:, None]
    rr -= np.rint(rr)
    wT = np.ascontiguousarray(rr.T).astype(np.float32)
    return {"xT": xT, "wT": wT}


def assemble_output(core_outs):
    full = np.empty((N, N), dtype=np.complex64)
    for c, r in enumerate(core_outs):
        oc = r["outT"].view(np.complex64)
        full[:, c * DSH : (c + 1) * DSH] = oc.T
    return full


_NC_CACHE = None


def get_nc():
    global _NC_CACHE
    if _NC_CACHE is None:
        _NC_CACHE = build_fft_nc()
    return _NC_CACHE


def make_in_maps(x: np.ndarray, weights: np.ndarray):
    x = np.asarray(x, dtype=np.float32)
    weights = np.asarray(weights, dtype=np.float32)
    in_maps = [make_core_inputs(x, weights, c) for c in range(NCORES)]
    return in_maps


def run_on_hw(x, weights, **spmd_kwargs):
    nc = get_nc()
    in_maps = make_in_maps(x, weights)
    res = run_bass_kernel_spmd(nc, in_maps, core_ids=list(range(NCORES)), **spmd_kwargs)
    return assemble_output(res.results), res


def kernel(x: np.ndarray, weights: np.ndarray) -> np.ndarray:
    out, _ = run_on_hw(x, weights)
    return out

